# revision 18
# baseline (speedup 1.0000x reference)
"""Trainium2 Bass kernel for nn_MultiHeadAttention_91027536871977.

Cosine-similarity multi-head self-attention:
  x      = einsum("bsd,hdf->bhsf", sin, Wx) + bx          [B,H,S,F]
  scores = (x @ x^T) / (|x| |x|^T)                        [B,H,S,S]
  p      = softmax(scores, -1)
  out    = concat_heads(p @ x) @ Wp + bp                  [B,S,D]

Sharding (v3): 8 cores = 4 batch-pairs x 2 head-halves.  Core c handles
batches {2*(c//2), 2*(c//2)+1} for heads [8*(c%2), 8*(c%2)+8).  Each core
runs two independent sub-problems (one per batch); the output projection
uses only the local 8 heads' rows of Wp, producing a partial Y in fp16.
The host sums the two cores' partials + bp (row-parallel Wp, host-side
reduce -> no on-device collectives).

Per-core pipeline, engineered so ScalarE (exp is 50%+ of the critical
path) sees a gapless stream:
  - X = sinT^T @ Wx + bx (bias via K=1 ones-row matmul in the same
    accumulation group); X^T per pair directly via Wx^T @ sinT matmuls
    plus the transposed bias outer product (no PE/DMA transposes of X)
  - 1/|x| per (row, head) via one batched DVE Newton-rsqrt chain per sub
    -> ScalarE's ACT table is loaded once (exp) and never switched
  - xn^T = bf16(X^T) * (1/|x| broadcast); the broadcast tiles come from
    fp16 selector matmuls, same machinery as the 1/rowsum broadcast
  - Gram per head-pair with K=64 row tiles; exp with accum_out rowsums
  - out^T = X^T E via col-packed K=128 matmuls; scaled by 1/rowsum
  - sub1's projection units and sub0's output-projection units are
    emitted as fillers inside the other sub's attention loop, and the
    prep/rowsum chains are deferred into the AV j-loop, so the pair
    boundary goes straight to the next pair's gram matmuls
  - PSUM: dedicated 2-deep gram pool (4 banks) + shared projection slot
    (2 banks) + AV accumulators (2 banks)

Measured on trn2 (8 cores, NTFF profile): ~290 us HW exec (run-to-run
~±10 us, HAM-phase dependent), scale-relative absmax error ~3.1e-3.
"""

import numpy as np
import ml_dtypes

import concourse.bass as bass
import concourse.bacc as bacc
import concourse.mybir as mybir
import concourse.tile as tile
from concourse.bass_utils import run_bass_kernel_spmd

B, S, D, H, F = 8, 1024, 1024, 16, 64
P = 128
HH = 8            # heads per core
NPAIR = HH // 2   # head pairs per sub-problem
NT = S // P       # s tiles
KO = D // P       # k subtiles for the d contraction
HW = HH * F       # 512: hf width per core
HALF = S // 2     # 512
BF16 = mybir.dt.bfloat16
F32 = mybir.dt.float32
F16 = mybir.dt.float16
MULT = mybir.AluOpType.mult
ADD = mybir.AluOpType.add
BYPASS = mybir.AluOpType.bypass


def build_program() -> bass.Bass:
    nc = bacc.Bacc("TRN2", target_bir_lowering=False, debug=False)

    d_sint = [nc.dram_tensor(f"sint{s}", [D, S], BF16, kind="ExternalInput")
              for s in range(2)]
    d_wx = nc.dram_tensor("wx", [D, HW], BF16, kind="ExternalInput")
    d_wp = nc.dram_tensor("wp", [HW, D], BF16, kind="ExternalInput")
    d_bx = nc.dram_tensor("bx", [1, HW], BF16, kind="ExternalInput")
    d_ones = nc.dram_tensor("ones", [1, HALF], BF16, kind="ExternalInput")
    d_sel8 = nc.dram_tensor("sel8", [2 * NT, NT, P], F16, kind="ExternalInput")
    d_ident = nc.dram_tensor("ident", [P, P], F16, kind="ExternalInput")
    d_y = [nc.dram_tensor(f"y{s}", [S, D], F16, kind="ExternalOutput")
           for s in range(2)]

    with tile.TileContext(nc) as tc:
        _body(tc, d_sint, d_wx, d_wp, d_bx, d_ones, d_sel8, d_ident, d_y)
    nc.compile()
    return nc


def _body(tc, d_sint, d_wx, d_wp, d_bx, d_ones, d_sel8, d_ident, d_y):
    nc = tc.nc
    from contextlib import ExitStack
    from collections import deque

    with ExitStack() as ctx:
        singles = ctx.enter_context(tc.tile_pool(name="singles", bufs=1))
        sq_pool = ctx.enter_context(tc.tile_pool(name="sq", bufs=2))
        nr_pool = ctx.enter_context(tc.tile_pool(name="nr", bufs=2))
        e_pool = ctx.enter_context(tc.tile_pool(name="epool", bufs=4))
        b_pool = ctx.enter_context(tc.tile_pool(name="bpool", bufs=2))
        brc_pool = ctx.enter_context(tc.tile_pool(name="brcpool", bufs=3))
        y_pool = ctx.enter_context(tc.tile_pool(name="ypool", bufs=2))

        # PSUM: gram gets a dedicated 2-deep pool (4 banks) so the exp
        # pipeline never competes with projections; everything else
        # shares ps_f (2 banks); AV accum + tiny transposes in ps_ot.
        ps_g = ctx.enter_context(tc.tile_pool(name="ps_g", bufs=2, space="PSUM"))
        ps_f = ctx.enter_context(tc.tile_pool(name="ps_f", bufs=1, space="PSUM"))
        ps_ot = ctx.enter_context(tc.tile_pool(name="ps_ot", bufs=2, space="PSUM"))

        # ---- load inputs to SBUF (sync queue only; scalar stays exp-only) --
        wx_sb = singles.tile([P, KO, HW], BF16)
        sint_sb = singles.tile([P, 2, KO, S], BF16)
        # split DMA issue across both HWDGE engines: each dma_start costs
        # ~0.7us of the issuing engine's sequencer, so one engine issuing
        # everything serializes the head
        wx_r = d_wx.rearrange("(ko p) n -> p ko n", p=P)
        for ko in range(KO):
            nc.scalar.dma_start(wx_sb[:, ko, :], wx_r[:, ko, :])
            nc.sync.dma_start(
                sint_sb[:, 0, ko, :],
                d_sint[0].rearrange("(ko p) s -> p ko s", p=P)[:, ko, :])
        ones_sb = singles.tile([1, HALF], BF16)
        nc.scalar.dma_start(ones_sb, d_ones[:, :])
        bx_sb = singles.tile([1, HW], BF16)
        nc.scalar.dma_start(bx_sb, d_bx[:, :])
        sel8_sb = singles.tile([2 * NT, NT, P], F16)
        nc.scalar.dma_start(sel8_sb, d_sel8[:, :, :])
        ident_sb = singles.tile([P, P], F16)
        nc.scalar.dma_start(ident_sb, d_ident[:, :])
        wp_sb = singles.tile([P, NPAIR, D], BF16)
        nc.scalar.dma_start(wp_sb, d_wp.rearrange("(q p) n -> p q n", p=P))
        for ko in range(KO):
            eng = nc.sync if ko % 2 == 0 else nc.scalar
            eng.dma_start(
                sint_sb[:, 1, ko, :],
                d_sint[1].rearrange("(ko p) s -> p ko s", p=P)[:, ko, :])

        # persistent intermediates
        x_sb = singles.tile([P, 2, NT, HW], BF16)      # raw x  [t, hf]
        xtn_sb = singles.tile([P, 2, NPAIR, S], BF16)  # xn^T [f2, pair, t]
        outt_sb = singles.tile([P, 2, NPAIR, S], BF16)
        rs_sb = singles.tile([P, 2, HH * NT], F32)     # rowsums [s_p, h*8+i]
        inv_sb = singles.tile([P, 2, HH * NT], F32)    # 1/|x|   [s_p, h*8+i]
        n2_sb = singles.tile([P, 2, HH * NT], F32)     # |x|^2   [s_p, h*8+i]

        es = {}

        def xproj_unit(s, i):
            """X tile i of sub s: matmul + bias, |x|^2, Newton 1/|x|."""
            x_ps = ps_f.tile([P, HW], F32, tag="f", name=f"xps{s}_{i}")
            for ko in range(KO):
                nc.tensor.matmul(
                    x_ps, lhsT=sint_sb[:, s, ko, i * P:(i + 1) * P],
                    rhs=wx_sb[:, ko, :], start=(ko == 0), stop=False)
            nc.tensor.matmul(x_ps, lhsT=ones_sb[0:1, 0:P], rhs=bx_sb[0:1, :],
                             start=False, stop=True)
            nc.vector.tensor_copy(x_sb[:, s, i, :], x_ps)
            xsq = sq_pool.tile([P, HW], BF16, tag="xsq", name=f"xsq{s}_{i}")
            nc.vector.tensor_mul(xsq, x_sb[:, s, i, :], x_sb[:, s, i, :])
            # reduce straight into the [s_p, h*NT + i] column layout
            n2_v = n2_sb.rearrange("p s (h i) -> p s h i", i=NT)[:, s, :, i]
            nc.vector.reduce_sum(
                n2_v, xsq.rearrange("p (h f) -> p h f", f=F),
                axis=mybir.AxisListType.X)

        def newton_sub(s):
            n2 = n2_sb[:, s, :]
            y = inv_sb[:, s, :]
            r_t = nr_pool.tile([P, HH * NT], F32, tag="nr_r", name=f"nr{s}")
            t_t = nr_pool.tile([P, HH * NT], F32, tag="nr_t", name=f"nt{s}")
            nc.vector.reciprocal(r_t, n2)
            nc.vector.tensor_scalar(y, r_t, 2.0, 0.105, MULT, ADD)
            for _ in range(3):
                nc.vector.tensor_mul(t_t, y, y)
                nc.vector.scalar_tensor_tensor(t_t, t_t, -0.5, n2, MULT, MULT)
                nc.vector.scalar_tensor_tensor(y, t_t, 1.5, y, ADD, MULT)

        def bcast16(s, q, src_cols, name, recip=False):
            """[128, 16] f32 cols (head-pair x tile layout) -> [f2, i, t_p]
            broadcast tiles via transpose + selector matmuls."""
            if recip:
                rc = b_pool.tile([P, 2 * NT], F32, tag=f"rc{name}",
                                 name=f"rc{name}{s}{q}")
                nc.vector.reciprocal(rc, src_cols)
                src_cols = rc
            cb = b_pool.tile([P, 2 * NT], F16, tag=f"cb{name}",
                             name=f"cb{name}{s}{q}")
            nc.vector.tensor_copy(cb, src_cols)
            tp = ps_ot.tile([2 * NT, P], F16, tag="ot", name=f"tp{name}{s}{q}")
            nc.tensor.transpose(tp, cb, ident_sb)
            tq = b_pool.tile([2 * NT, P], F16, tag=f"tq{name}",
                             name=f"tq{name}{s}{q}")
            nc.vector.tensor_copy(tq, tp)
            bc_ps = ps_f.tile([P, NT * P], F32, tag="f", name=f"bp{name}{s}{q}")
            for i in range(NT):
                nc.tensor.matmul(bc_ps[:, i * P:(i + 1) * P],
                                 lhsT=sel8_sb[:, i, :], rhs=tq,
                                 start=True, stop=True)
            bc = brc_pool.tile([P, NT, P], F16, tag="brc", name=f"bc{name}{s}{q}")
            nc.vector.tensor_copy(bc.rearrange("p a b -> p (a b)"), bc_ps)
            return bc

        def prep_pair(s, q):
            """xn^T for pair q of sub s: 1/|x| broadcast + direct X^T matmul
            with normalization fused into the PSUM->SBUF copy."""
            nrm = bcast16(s, q, inv_sb[:, s, q * 2 * NT:(q + 1) * 2 * NT], "n")
            xt_ps = ps_f.tile([P, S], F32, tag="f", name=f"xtp{s}{q}")
            for hlf in range(2):
                for ko in range(KO):
                    nc.tensor.matmul(
                        xt_ps[:, hlf * HALF:(hlf + 1) * HALF],
                        lhsT=wx_sb[:, ko, q * P:(q + 1) * P],
                        rhs=sint_sb[:, s, ko, hlf * HALF:(hlf + 1) * HALF],
                        start=(ko == 0), stop=False)
                # transposed bias: xt[f2, t] += bx[f2] (scores must see the
                # same biased x the AV path uses)
                nc.tensor.matmul(
                    xt_ps[:, hlf * HALF:(hlf + 1) * HALF],
                    lhsT=bx_sb[0:1, q * P:(q + 1) * P],
                    rhs=ones_sb[0:1, :],
                    start=False, stop=True)
            # round x^T to bf16 BEFORE normalizing so the Gram sees exactly
            # the same x values as the AV stationary (x_sb): the 1/|x| factor
            # is derived from bf16 x, and normalizing fp32 x with it injects
            # ~0.4% inconsistency into every score
            xtr = sq_pool.tile([P, S], BF16, tag="xtr", name=f"xtr{s}{q}")
            nc.vector.tensor_copy(xtr, xt_ps)
            nc.vector.tensor_mul(
                xtn_sb[:, s, q, :], xtr,
                nrm.rearrange("p a b -> p (a b)"))

        def gram_tile(s, q, i):
            """Cosine-score Gram + exp for both heads of pair q, s-tile i."""
            g = [ps_g.tile([P, S], F32, tag="g", name=f"g{s}{q}{hh}_{i}")
                 for hh in range(2)]
            for hh in range(2):
                for hlf in range(2):
                    nc.tensor.matmul(
                        g[hh][:, hlf * HALF:(hlf + 1) * HALF],
                        lhsT=xtn_sb[hh * F:(hh + 1) * F, s, q, i * P:(i + 1) * P],
                        rhs=xtn_sb[hh * F:(hh + 1) * F, s, q,
                                   hlf * HALF:(hlf + 1) * HALF],
                        start=True, stop=True, tile_position=(hh * F, 0))
                h = 2 * q + hh
                nc.scalar.activation(
                    es[(s, q)][hh][:, i, :], g[hh],
                    mybir.ActivationFunctionType.Exp,
                    accum_out=rs_sb[:, s, h * NT + i:h * NT + i + 1])

        def ex_half(s, q, hlf, nxt, pop_js, fillers, jobs=None):
            ot = ps_ot.tile([P, HALF], F32, tag="ot", name=f"ot{s}{q}{hlf}")
            for j in range(NT):
                if nxt is not None and j % 2 == 0:
                    gram_tile(nxt[0], nxt[1], hlf * 4 + j // 2)
                if jobs and j in jobs:
                    jobs[j]()
                if j in pop_js and fillers:
                    fillers.popleft()()
                for hh in range(2):
                    nc.tensor.matmul(
                        ot[hh * F:(hh + 1) * F, :],
                        lhsT=x_sb[:, s, j, (2 * q + hh) * F:(2 * q + hh + 1) * F],
                        rhs=es[(s, q)][hh][:, j, hlf * HALF:(hlf + 1) * HALF],
                        start=(j == 0), stop=(j == NT - 1),
                        tile_position=(0, hh * F), skip_group_check=True)
            return ot

        def yproj_unit(s, i):
            y_ps = ps_f.tile([P, D], F32, tag="f", name=f"yps{s}_{i}")
            for hlf in range(2):
                for q in range(NPAIR):
                    nc.tensor.matmul(
                        y_ps[:, hlf * HALF:(hlf + 1) * HALF],
                        lhsT=outt_sb[:, s, q, i * P:(i + 1) * P],
                        rhs=wp_sb[:, q, hlf * HALF:(hlf + 1) * HALF],
                        start=(q == 0), stop=(q == NPAIR - 1))
            ysb = y_pool.tile([P, D], F16, tag="y", name=f"ysb{s}_{i}")
            if s == 1:
                # tail: exp is done, ScalarE is idle -> PSUM->fp16 copies there
                # (ACT Copy is in every table set: no table switch)
                nc.scalar.copy(ysb, y_ps)
            else:
                nc.vector.tensor_copy(ysb, y_ps)
            nc.sync.dma_start(d_y[s][i * P:(i + 1) * P, :], ysb)

        # ---- head: sub0 projections, pair-(0,0) prep, seed grams ----
        for i in range(NT):
            xproj_unit(0, i)
        newton_sub(0)
        prep_pair(0, 0)
        es[(0, 0)] = [e_pool.tile([P, NT, S], BF16, tag="e", name=f"e00{hh}")
                      for hh in range(2)]
        for i in range(NT):
            gram_tile(0, 0, i)
        prep_pair(0, 1)

        fillers = deque([(lambda ii: (lambda: xproj_unit(1, ii)))(i)
                         for i in range(NT)])
        pair_seq = [(0, q) for q in range(NPAIR)] + [(1, q) for q in range(NPAIR)]
        # halves 0-3: sub1 xproj (2 per half); halves 8-15: sub0 yproj
        half_quota = {0: 2, 1: 2, 2: 2, 3: 2,
                      8: 1, 9: 1, 10: 1, 11: 1, 12: 1, 13: 1, 14: 1, 15: 1}
        for idx, (s, q) in enumerate(pair_seq):
            if idx == NPAIR:
                fillers.extend([(lambda ii: (lambda: yproj_unit(0, ii)))(i)
                                for i in range(NT)])
            nxt = pair_seq[idx + 1] if idx + 1 < len(pair_seq) else None
            if nxt is not None:
                es[nxt] = [e_pool.tile([P, NT, S], BF16,
                                       tag="e", name=f"e{nxt[0]}{nxt[1]}{hh}")
                           for hh in range(2)]
            # deferred jobs run INSIDE the hlf0 j-loop so the pair boundary
            # goes straight to gram(nxt) matmuls (keeps the exp stream fed):
            #   j==1: prep for pair idx+2 (+ sub1's batched newton first)
            #   j==3: this pair's 1/rowsum broadcast chain (exps already done
            #         for idx>0; for pair 0 it must wait its own exps anyway)
            state = {}

            def mk_prep():
                if idx == 2:
                    newton_sub(1)
                if idx + 2 < len(pair_seq):
                    prep_pair(*pair_seq[idx + 2])

            def mk_brc():
                state["brc"] = bcast16(
                    s, q, rs_sb[:, s, q * 2 * NT:(q + 1) * 2 * NT],
                    "r", recip=True)

            for hlf in range(2):
                quota = half_quota.get(idx * 2 + hlf, 0)
                pop_js = {2, 6} if quota >= 2 else ({6} if quota == 1 else set())
                jobs = {1: mk_prep, 3: mk_brc} if hlf == 0 else None
                ot = ex_half(s, q, hlf, nxt, pop_js, fillers, jobs)
                nc.vector.tensor_mul(
                    outt_sb[:, s, q, hlf * HALF:(hlf + 1) * HALF],
                    state["brc"].rearrange(
                        "p a b -> p (a b)")[:, hlf * HALF:(hlf + 1) * HALF],
                    ot)
            del es[(s, q)]

        # ---- tail: sub1 output projection ----
        for i in range(NT):
            yproj_unit(1, i)


_CACHE: dict = {}


def _get_program() -> bass.Bass:
    if "nc" not in _CACHE:
        _CACHE["nc"] = build_program()
    return _CACHE["nc"]


def _prep_inputs(sin, Wx, bx, Wp, bp):
    bf16 = ml_dtypes.bfloat16
    sin32 = np.asarray(sin, np.float32)
    wx32 = np.asarray(Wx, np.float32)          # [H, D, F]
    wp32 = np.asarray(Wp, np.float32)          # [H*F, D]
    bx32 = np.asarray(bx, np.float32)          # [H, F]
    sel8 = np.zeros((2 * NT, NT, P), np.float32)
    for i in range(NT):
        sel8[i, i, :F] = 1.0
        sel8[NT + i, i, F:] = 1.0
    sel8 = sel8.astype(np.float16)
    ident = np.eye(P, dtype=np.float32).astype(np.float16)
    ones = np.ones((1, HALF), np.float32).astype(bf16)
    sints = [np.ascontiguousarray(sin32[b].T).astype(bf16) for b in range(B)]
    in_maps = []
    for c in range(B):
        pi, hg = c // 2, c % 2
        wxh = np.ascontiguousarray(
            np.transpose(wx32[hg * HH:(hg + 1) * HH], (1, 0, 2)).reshape(D, HW)
        ).astype(bf16)
        wph = np.ascontiguousarray(wp32[hg * HW:(hg + 1) * HW, :]).astype(bf16)
        bxh = np.ascontiguousarray(
            bx32[hg * HH:(hg + 1) * HH].reshape(1, HW)).astype(bf16)
        in_maps.append({
            "sint0": sints[2 * pi], "sint1": sints[2 * pi + 1],
            "wx": wxh, "wp": wph, "bx": bxh, "ones": ones,
            "sel8": sel8, "ident": ident,
        })
    return in_maps


def kernel(sin, mask, Wx, bx, Wp, bp, _run_kwargs=None):
    nc = _get_program()
    in_maps = _prep_inputs(sin, Wx, bx, Wp, bp)
    res = run_bass_kernel_spmd(nc, in_maps, core_ids=list(range(B)),
                               **(_run_kwargs or {}))
    bp32 = np.asarray(bp, np.float32)
    out = np.empty((B, S, D), np.float32)
    for b in range(B):
        pi, s = b // 2, b % 2
        p0 = np.asarray(res.results[2 * pi][f"y{s}"], np.float32)
        p1 = np.asarray(res.results[2 * pi + 1][f"y{s}"], np.float32)
        out[b] = p0 + p1 + bp32
    if _run_kwargs:
        _CACHE["last_results"] = res
    return out


# revision 19
# speedup vs baseline: 1.0273x; 1.0273x over previous
"""Trainium2 Bass kernel for nn_MultiHeadAttention_91027536871977.

Cosine-similarity multi-head self-attention:
  x      = einsum("bsd,hdf->bhsf", sin, Wx) + bx          [B,H,S,F]
  scores = (x @ x^T) / (|x| |x|^T)                        [B,H,S,S]
  p      = softmax(scores, -1)
  out    = concat_heads(p @ x) @ Wp + bp                  [B,S,D]

Sharding (v3): 8 cores = 4 batch-pairs x 2 head-halves.  Core c handles
batches {2*(c//2), 2*(c//2)+1} for heads [8*(c%2), 8*(c%2)+8).  Each core
runs two independent sub-problems (one per batch); the output projection
uses only the local 8 heads' rows of Wp, producing a partial Y in fp16.
The host sums the two cores' partials + bp (row-parallel Wp, host-side
reduce -> no on-device collectives).

Per-core pipeline, engineered so ScalarE (exp is 50%+ of the critical
path) sees a gapless stream:
  - X = sinT^T @ Wx + bx (bias via K=1 ones-row matmul in the same
    accumulation group); X^T per pair directly via Wx^T @ sinT matmuls
    plus the transposed bias outer product (no PE/DMA transposes of X)
  - 1/|x| per (row, head) via one batched DVE Newton-rsqrt chain per sub
    -> ScalarE's ACT table is loaded once (exp) and never switched
  - xn^T = bf16(X^T) * (1/|x| broadcast); the broadcast tiles come from
    fp16 selector matmuls, same machinery as the 1/rowsum broadcast
  - Gram per head-pair with K=64 row tiles; exp with accum_out rowsums
  - out^T = X^T E via col-packed K=128 matmuls; scaled by 1/rowsum
  - sub1's projection units and sub0's output-projection units are
    emitted as fillers inside the other sub's attention loop, and the
    prep/rowsum chains are deferred into the AV j-loop, so the pair
    boundary goes straight to the next pair's gram matmuls
  - PSUM: dedicated 2-deep gram pool (4 banks) + shared projection slot
    (2 banks) + AV accumulators (2 banks)

Measured on trn2 (8 cores, NTFF profile): ~290 us HW exec (run-to-run
~±10 us, HAM-phase dependent), scale-relative absmax error ~3.1e-3.
"""

import numpy as np
import ml_dtypes

import concourse.bass as bass
import concourse.bacc as bacc
import concourse.mybir as mybir
import concourse.tile as tile
from concourse.bass_utils import run_bass_kernel_spmd

B, S, D, H, F = 8, 1024, 1024, 16, 64
P = 128
HH = 8            # heads per core
NPAIR = HH // 2   # head pairs per sub-problem
NT = S // P       # s tiles
KO = D // P       # k subtiles for the d contraction
HW = HH * F       # 512: hf width per core
HALF = S // 2     # 512
BF16 = mybir.dt.bfloat16
F32 = mybir.dt.float32
F16 = mybir.dt.float16
MULT = mybir.AluOpType.mult
ADD = mybir.AluOpType.add
BYPASS = mybir.AluOpType.bypass


def build_program() -> bass.Bass:
    nc = bacc.Bacc("TRN2", target_bir_lowering=False, debug=False)

    d_sint = [nc.dram_tensor(f"sint{s}", [D, S], BF16, kind="ExternalInput")
              for s in range(2)]
    d_wx = nc.dram_tensor("wx", [D, HW], BF16, kind="ExternalInput")
    d_wp = nc.dram_tensor("wp", [HW, D], BF16, kind="ExternalInput")
    d_bx = nc.dram_tensor("bx", [1, HW], BF16, kind="ExternalInput")
    d_ones = nc.dram_tensor("ones", [1, HALF], BF16, kind="ExternalInput")
    d_sel8 = nc.dram_tensor("sel8", [2 * NT, NT, P], F16, kind="ExternalInput")
    d_ident = nc.dram_tensor("ident", [P, P], F16, kind="ExternalInput")
    d_y = [nc.dram_tensor(f"y{s}", [S, D], F16, kind="ExternalOutput")
           for s in range(2)]

    with tile.TileContext(nc) as tc:
        _body(tc, d_sint, d_wx, d_wp, d_bx, d_ones, d_sel8, d_ident, d_y)
    nc.compile()
    return nc


def _body(tc, d_sint, d_wx, d_wp, d_bx, d_ones, d_sel8, d_ident, d_y):
    nc = tc.nc
    from contextlib import ExitStack
    from collections import deque

    with ExitStack() as ctx:
        singles = ctx.enter_context(tc.tile_pool(name="singles", bufs=1))
        sq_pool = ctx.enter_context(tc.tile_pool(name="sq", bufs=2))
        nr_pool = ctx.enter_context(tc.tile_pool(name="nr", bufs=2))
        e_pool = ctx.enter_context(tc.tile_pool(name="epool", bufs=4))
        b_pool = ctx.enter_context(tc.tile_pool(name="bpool", bufs=2))
        brc_pool = ctx.enter_context(tc.tile_pool(name="brcpool", bufs=3))
        y_pool = ctx.enter_context(tc.tile_pool(name="ypool", bufs=2))

        # PSUM: gram gets a dedicated 2-deep pool (4 banks) so the exp
        # pipeline never competes with projections; everything else
        # shares ps_f (2 banks); AV accum + tiny transposes in ps_ot.
        ps_g = ctx.enter_context(tc.tile_pool(name="ps_g", bufs=2, space="PSUM"))
        ps_f = ctx.enter_context(tc.tile_pool(name="ps_f", bufs=1, space="PSUM"))
        ps_ot = ctx.enter_context(tc.tile_pool(name="ps_ot", bufs=2, space="PSUM"))

        # ---- load inputs to SBUF (sync queue only; scalar stays exp-only) --
        wx_sb = singles.tile([P, KO, HW], BF16)
        sint_sb = singles.tile([P, 2, KO, S], BF16)
        # split DMA issue across both HWDGE engines: each dma_start costs
        # ~0.7us of the issuing engine's sequencer, so one engine issuing
        # everything serializes the head
        wx_r = d_wx.rearrange("(ko p) n -> p ko n", p=P)
        for ko in range(KO):
            nc.scalar.dma_start(wx_sb[:, ko, :], wx_r[:, ko, :])
            nc.sync.dma_start(
                sint_sb[:, 0, ko, :],
                d_sint[0].rearrange("(ko p) s -> p ko s", p=P)[:, ko, :])
        ones_sb = singles.tile([1, HALF], BF16)
        nc.scalar.dma_start(ones_sb, d_ones[:, :])
        bx_sb = singles.tile([1, HW], BF16)
        nc.scalar.dma_start(bx_sb, d_bx[:, :])
        sel8_sb = singles.tile([2 * NT, NT, P], F16)
        nc.scalar.dma_start(sel8_sb, d_sel8[:, :, :])
        ident_sb = singles.tile([P, P], F16)
        nc.scalar.dma_start(ident_sb, d_ident[:, :])
        wp_sb = singles.tile([P, NPAIR, D], BF16)
        nc.scalar.dma_start(wp_sb, d_wp.rearrange("(q p) n -> p q n", p=P))
        for ko in range(KO):
            eng = nc.sync if ko % 2 == 0 else nc.scalar
            eng.dma_start(
                sint_sb[:, 1, ko, :],
                d_sint[1].rearrange("(ko p) s -> p ko s", p=P)[:, ko, :])

        # persistent intermediates
        x_sb = singles.tile([P, 2, NT, HW], BF16)      # raw x  [t, hf]
        xtn_sb = singles.tile([P, 2, NPAIR, S], BF16)  # xn^T [f2, pair, t]
        outt_sb = singles.tile([P, 2, NPAIR, S], BF16)
        rs_sb = singles.tile([P, 2, HH * NT], F32)     # rowsums [s_p, h*8+i]
        inv_sb = singles.tile([P, 2, HH * NT], F32)    # 1/|x|   [s_p, h*8+i]
        n2_sb = singles.tile([P, 2, HH * NT], F32)     # |x|^2   [s_p, h*8+i]

        es = {}

        def xproj_unit(s, i):
            """X tile i of sub s: matmul + bias, |x|^2, Newton 1/|x|."""
            x_ps = ps_f.tile([P, HW], F32, tag="f", name=f"xps{s}_{i}")
            for ko in range(KO):
                nc.tensor.matmul(
                    x_ps, lhsT=sint_sb[:, s, ko, i * P:(i + 1) * P],
                    rhs=wx_sb[:, ko, :], start=(ko == 0), stop=False)
            nc.tensor.matmul(x_ps, lhsT=ones_sb[0:1, 0:P], rhs=bx_sb[0:1, :],
                             start=False, stop=True)
            nc.vector.tensor_copy(x_sb[:, s, i, :], x_ps)
            xsq = sq_pool.tile([P, HW], BF16, tag="xsq", name=f"xsq{s}_{i}")
            nc.vector.tensor_mul(xsq, x_sb[:, s, i, :], x_sb[:, s, i, :])
            # reduce straight into the [s_p, h*NT + i] column layout
            n2_v = n2_sb.rearrange("p s (h i) -> p s h i", i=NT)[:, s, :, i]
            nc.vector.reduce_sum(
                n2_v, xsq.rearrange("p (h f) -> p h f", f=F),
                axis=mybir.AxisListType.X)

        def newton_sub(s):
            n2 = n2_sb[:, s, :]
            y = inv_sb[:, s, :]
            r_t = nr_pool.tile([P, HH * NT], F32, tag="nr_r", name=f"nr{s}")
            t_t = nr_pool.tile([P, HH * NT], F32, tag="nr_t", name=f"nt{s}")
            nc.vector.reciprocal(r_t, n2)
            nc.vector.tensor_scalar(y, r_t, 2.0, 0.105, MULT, ADD)
            for _ in range(3):
                nc.vector.tensor_mul(t_t, y, y)
                nc.vector.scalar_tensor_tensor(t_t, t_t, -0.5, n2, MULT, MULT)
                nc.vector.scalar_tensor_tensor(y, t_t, 1.5, y, ADD, MULT)

        def bcast16(s, q, src_cols, name, recip=False):
            """[128, 16] f32 cols (head-pair x tile layout) -> [f2, i, t_p]
            broadcast tiles via transpose + selector matmuls."""
            if recip:
                rc = b_pool.tile([P, 2 * NT], F32, tag=f"rc{name}",
                                 name=f"rc{name}{s}{q}")
                nc.vector.reciprocal(rc, src_cols)
                src_cols = rc
            cb = b_pool.tile([P, 2 * NT], F16, tag=f"cb{name}",
                             name=f"cb{name}{s}{q}")
            nc.vector.tensor_copy(cb, src_cols)
            tp = ps_ot.tile([2 * NT, P], F16, tag="ot", name=f"tp{name}{s}{q}")
            nc.tensor.transpose(tp, cb, ident_sb)
            tq = b_pool.tile([2 * NT, P], F16, tag=f"tq{name}",
                             name=f"tq{name}{s}{q}")
            nc.vector.tensor_copy(tq, tp)
            bc_ps = ps_f.tile([P, NT * P], F32, tag="f", name=f"bp{name}{s}{q}")
            for i in range(NT):
                nc.tensor.matmul(bc_ps[:, i * P:(i + 1) * P],
                                 lhsT=sel8_sb[:, i, :], rhs=tq,
                                 start=True, stop=True)
            bc = brc_pool.tile([P, NT, P], F16, tag="brc", name=f"bc{name}{s}{q}")
            nc.vector.tensor_copy(bc.rearrange("p a b -> p (a b)"), bc_ps)
            return bc

        def prep_pair(s, q):
            """xn^T for pair q of sub s: 1/|x| broadcast + direct X^T matmul
            with normalization fused into the PSUM->SBUF copy."""
            nrm = bcast16(s, q, inv_sb[:, s, q * 2 * NT:(q + 1) * 2 * NT], "n")
            xt_ps = ps_f.tile([P, S], F32, tag="f", name=f"xtp{s}{q}")
            for hlf in range(2):
                for ko in range(KO):
                    nc.tensor.matmul(
                        xt_ps[:, hlf * HALF:(hlf + 1) * HALF],
                        lhsT=wx_sb[:, ko, q * P:(q + 1) * P],
                        rhs=sint_sb[:, s, ko, hlf * HALF:(hlf + 1) * HALF],
                        start=(ko == 0), stop=False)
                # transposed bias: xt[f2, t] += bx[f2] (scores must see the
                # same biased x the AV path uses)
                nc.tensor.matmul(
                    xt_ps[:, hlf * HALF:(hlf + 1) * HALF],
                    lhsT=bx_sb[0:1, q * P:(q + 1) * P],
                    rhs=ones_sb[0:1, :],
                    start=False, stop=True)
            # round x^T to bf16 BEFORE normalizing so the Gram sees exactly
            # the same x values as the AV stationary (x_sb): the 1/|x| factor
            # is derived from bf16 x, and normalizing fp32 x with it injects
            # ~0.4% inconsistency into every score
            xtr = sq_pool.tile([P, S], BF16, tag="xtr", name=f"xtr{s}{q}")
            nc.vector.tensor_copy(xtr, xt_ps)
            nc.vector.tensor_mul(
                xtn_sb[:, s, q, :], xtr,
                nrm.rearrange("p a b -> p (a b)"))

        def gram_tile(s, q, i):
            """Cosine-score Gram + exp for both heads of pair q, s-tile i."""
            g = [ps_g.tile([P, S], F32, tag="g", name=f"g{s}{q}{hh}_{i}")
                 for hh in range(2)]
            for hh in range(2):
                for hlf in range(2):
                    nc.tensor.matmul(
                        g[hh][:, hlf * HALF:(hlf + 1) * HALF],
                        lhsT=xtn_sb[hh * F:(hh + 1) * F, s, q, i * P:(i + 1) * P],
                        rhs=xtn_sb[hh * F:(hh + 1) * F, s, q,
                                   hlf * HALF:(hlf + 1) * HALF],
                        start=True, stop=True, tile_position=(hh * F, 0))
                h = 2 * q + hh
                nc.scalar.activation(
                    es[(s, q)][hh][:, i, :], g[hh],
                    mybir.ActivationFunctionType.Exp,
                    accum_out=rs_sb[:, s, h * NT + i:h * NT + i + 1])

        def ex_half(s, q, hlf, nxt, pop_js, fillers, jobs=None):
            ot = ps_ot.tile([P, HALF], F32, tag="ot", name=f"ot{s}{q}{hlf}")
            for j in range(NT):
                if nxt is not None and j % 2 == 0:
                    gram_tile(nxt[0], nxt[1], hlf * 4 + j // 2)
                if jobs and j in jobs:
                    jobs[j]()
                if j in pop_js and fillers:
                    fillers.popleft()()
                for hh in range(2):
                    nc.tensor.matmul(
                        ot[hh * F:(hh + 1) * F, :],
                        lhsT=x_sb[:, s, j, (2 * q + hh) * F:(2 * q + hh + 1) * F],
                        rhs=es[(s, q)][hh][:, j, hlf * HALF:(hlf + 1) * HALF],
                        start=(j == 0), stop=(j == NT - 1),
                        tile_position=(0, hh * F), skip_group_check=True)
            return ot

        def yproj_unit(s, i):
            # in the tail (s==1) the gram pool is free: double-buffer the
            # projection PSUM across both pools so the PE never waits for a
            # cast to release the single shared slot
            if s == 1 and i % 2 == 1:
                y_ps = ps_g.tile([P, D], F32, tag="g", name=f"yps{s}_{i}")
            else:
                y_ps = ps_f.tile([P, D], F32, tag="f", name=f"yps{s}_{i}")
            for hlf in range(2):
                for q in range(NPAIR):
                    nc.tensor.matmul(
                        y_ps[:, hlf * HALF:(hlf + 1) * HALF],
                        lhsT=outt_sb[:, s, q, i * P:(i + 1) * P],
                        rhs=wp_sb[:, q, hlf * HALF:(hlf + 1) * HALF],
                        start=(q == 0), stop=(q == NPAIR - 1))
            ysb = y_pool.tile([P, D], F16, tag="y", name=f"ysb{s}_{i}")
            if s == 1 and i % 2 == 0:
                # tail: exp is done, ScalarE is idle -> alternate the
                # PSUM->fp16 copies between ScalarE and DVE
                # (ACT Copy is in every table set: no table switch)
                nc.scalar.copy(ysb, y_ps)
            else:
                nc.vector.tensor_copy(ysb, y_ps)
            nc.sync.dma_start(d_y[s][i * P:(i + 1) * P, :], ysb)

        # ---- head: sub0 projections, pair-(0,0) prep, seed grams ----
        for i in range(NT):
            xproj_unit(0, i)
        newton_sub(0)
        prep_pair(0, 0)
        es[(0, 0)] = [e_pool.tile([P, NT, S], BF16, tag="e", name=f"e00{hh}")
                      for hh in range(2)]
        for i in range(NT):
            gram_tile(0, 0, i)
        prep_pair(0, 1)

        fillers = deque([(lambda ii: (lambda: xproj_unit(1, ii)))(i)
                         for i in range(NT)])
        pair_seq = [(0, q) for q in range(NPAIR)] + [(1, q) for q in range(NPAIR)]
        # halves 0-3: sub1 xproj (2 per half); halves 8-15: sub0 yproj
        half_quota = {0: 2, 1: 2, 2: 2, 3: 2,
                      8: 1, 9: 1, 10: 1, 11: 1, 12: 1, 13: 1, 14: 1, 15: 1}
        for idx, (s, q) in enumerate(pair_seq):
            if idx == NPAIR:
                fillers.extend([(lambda ii: (lambda: yproj_unit(0, ii)))(i)
                                for i in range(NT)])
            nxt = pair_seq[idx + 1] if idx + 1 < len(pair_seq) else None
            if nxt is not None:
                es[nxt] = [e_pool.tile([P, NT, S], BF16,
                                       tag="e", name=f"e{nxt[0]}{nxt[1]}{hh}")
                           for hh in range(2)]
            # deferred jobs run INSIDE the hlf0 j-loop so the pair boundary
            # goes straight to gram(nxt) matmuls (keeps the exp stream fed):
            #   j==1: prep for pair idx+2 (+ sub1's batched newton first)
            #   j==3: this pair's 1/rowsum broadcast chain (exps already done
            #         for idx>0; for pair 0 it must wait its own exps anyway)
            state = {}

            def mk_prep():
                if idx == 2:
                    newton_sub(1)
                if idx + 2 < len(pair_seq):
                    prep_pair(*pair_seq[idx + 2])

            def mk_brc():
                state["brc"] = bcast16(
                    s, q, rs_sb[:, s, q * 2 * NT:(q + 1) * 2 * NT],
                    "r", recip=True)

            for hlf in range(2):
                quota = half_quota.get(idx * 2 + hlf, 0)
                pop_js = {2, 6} if quota >= 2 else ({6} if quota == 1 else set())
                jobs = {1: mk_prep, 3: mk_brc} if hlf == 0 else None
                ot = ex_half(s, q, hlf, nxt, pop_js, fillers, jobs)
                nc.vector.tensor_mul(
                    outt_sb[:, s, q, hlf * HALF:(hlf + 1) * HALF],
                    state["brc"].rearrange(
                        "p a b -> p (a b)")[:, hlf * HALF:(hlf + 1) * HALF],
                    ot)
            del es[(s, q)]

        # ---- tail: sub1 output projection ----
        for i in range(NT):
            yproj_unit(1, i)


_CACHE: dict = {}


def _get_program() -> bass.Bass:
    if "nc" not in _CACHE:
        _CACHE["nc"] = build_program()
    return _CACHE["nc"]


def _prep_inputs(sin, Wx, bx, Wp, bp):
    bf16 = ml_dtypes.bfloat16
    sin32 = np.asarray(sin, np.float32)
    wx32 = np.asarray(Wx, np.float32)          # [H, D, F]
    wp32 = np.asarray(Wp, np.float32)          # [H*F, D]
    bx32 = np.asarray(bx, np.float32)          # [H, F]
    sel8 = np.zeros((2 * NT, NT, P), np.float32)
    for i in range(NT):
        sel8[i, i, :F] = 1.0
        sel8[NT + i, i, F:] = 1.0
    sel8 = sel8.astype(np.float16)
    ident = np.eye(P, dtype=np.float32).astype(np.float16)
    ones = np.ones((1, HALF), np.float32).astype(bf16)
    sints = [np.ascontiguousarray(sin32[b].T).astype(bf16) for b in range(B)]
    in_maps = []
    for c in range(B):
        pi, hg = c // 2, c % 2
        wxh = np.ascontiguousarray(
            np.transpose(wx32[hg * HH:(hg + 1) * HH], (1, 0, 2)).reshape(D, HW)
        ).astype(bf16)
        wph = np.ascontiguousarray(wp32[hg * HW:(hg + 1) * HW, :]).astype(bf16)
        bxh = np.ascontiguousarray(
            bx32[hg * HH:(hg + 1) * HH].reshape(1, HW)).astype(bf16)
        in_maps.append({
            "sint0": sints[2 * pi], "sint1": sints[2 * pi + 1],
            "wx": wxh, "wp": wph, "bx": bxh, "ones": ones,
            "sel8": sel8, "ident": ident,
        })
    return in_maps


def kernel(sin, mask, Wx, bx, Wp, bp, _run_kwargs=None):
    nc = _get_program()
    in_maps = _prep_inputs(sin, Wx, bx, Wp, bp)
    res = run_bass_kernel_spmd(nc, in_maps, core_ids=list(range(B)),
                               **(_run_kwargs or {}))
    bp32 = np.asarray(bp, np.float32)
    out = np.empty((B, S, D), np.float32)
    for b in range(B):
        pi, s = b // 2, b % 2
        p0 = np.asarray(res.results[2 * pi][f"y{s}"], np.float32)
        p1 = np.asarray(res.results[2 * pi + 1][f"y{s}"], np.float32)
        out[b] = p0 + p1 + bp32
    if _run_kwargs:
        _CACHE["last_results"] = res
    return out
